# revision 2
# baseline (speedup 1.0000x reference)
"""Differentiable rasterizer on 8 Trainium2 NeuronCores (Bass/Tile), v2.

Math identical to v1: per pixel/stroke, min over bezier samples of squared
distance via a TensorEngine quadratic form; alpha compositing in closed form
    C = 1 + sum_s alpha_s * T_s * (c_s - 1),  T_s = prod_{j>s} (1 - alpha_j)
in log space with a triangular matmul, using only the ln/exp ACT table:
    2d   = exp(0.5 * ln(-4*m))
    sp   = softplus(2w - 2d) = ln(1 + exp(2w - 2d))
    w_s  = exp((2w - 2d) - sp + psumE) = exp(pE'' - 2d)
where pE'' = (U-I) @ sp + w2 comes from ONE matmul (w2 enters via an
appended ones-row in the rhs and a w2 row in the matrix).

v2 changes for speed:
  - W_MARGIN 46 -> 8 (alpha < sigmoid(-16) ~ 1e-7 beyond; tolerance 2e-2).
    Tiles with no active stroke are skipped on device; host fills 1.0.
  - All matmuls in fp16 (1 PE cycle/row vs fp32's 4). Candidate geometry is
    kept exact with hi/lo split rows (K=8 quadratic form).
  - Variable-depth partition packing: a block stacks as many tiles as fit in
    127 partition rows (strokes) x 128 tile-local pixel columns.
  - Per-chunk candidate padding (chunks are stroke runs with uniform k),
    no segment/max-combine passes.
  - Stage-A per-stroke min clamped to -DELTA on the (idle) Pool engine so
    Ln never sees a non-negative input (pixel exactly on a sample).
"""
import os
import sys
import time

import numpy as np

sys.path.insert(0, "/opt/trn_rl_repo")

import concourse.bass as bass
import concourse.mybir as mybir
from concourse.tile import TileContext
from concourse.bass_utils import run_bass_kernel_spmd

AF = mybir.ActivationFunctionType
ALU = mybir.AluOpType
F32 = mybir.dt.float32
F16 = mybir.dt.float16

CS = 512
NSAMP = 50
NSTR = 64
TH, TW = 8, 16
NTY, NTX = CS // TH, CS // TW
NCORES = 8
R_TILE = float(np.hypot((TW - 1) / 2.0, (TH - 1) / 2.0))
W_MARGIN = 8.0
DELTA = 2.0e-3  # clamp: m <= -DELTA so ln(-4m) is finite
DUMMY_N = -60000.0  # dummy candidate norm row (fits fp16)
PAD_MB = -1000.0  # memset value for mb pad columns

MAX_WAITS = 1
MGMAX = 8  # max blocks per mega-group (mT psum = MGMAX*128 fp16 cols)
CGMAX = 4  # max blocks per stage-B chunk group
PMAX = 128  # strokes per block
MMAX = 42  # tiles per block (3*m <= 126 partition rows in pC)


def _split_excess_waits(nc):
    """walrus rejects >1 sync-wait per instruction; move extras onto NoOps."""
    n_split = 0
    for fn in nc.m.functions:
        for bb in fn.blocks:
            insts = list(bb.instructions)
            out = []
            changed = False
            for inst in insts:
                si = inst.sync_info
                waits = list(si.on_wait) if si is not None and si.on_wait else []
                if len(waits) > MAX_WAITS:
                    changed = True
                    extra = waits[: len(waits) - MAX_WAITS]
                    keep = waits[len(extra):]
                    for i in range(0, len(extra), MAX_WAITS):
                        nop = mybir.InstNoOp(
                            name=f"{inst.name}-ws{n_split}-{i}", ins=[], outs=[]
                        )
                        nop.engine = inst.engine
                        nop.sync_info = mybir.SyncInfo(
                            on_wait=extra[i : i + MAX_WAITS], on_update=[]
                        )
                        out.append(nop)
                    si.on_wait = keep
                    n_split += 1
                out.append(inst)
            if changed:
                bb.instructions[:] = out
    return n_split


def _sample_points(strokes):
    """Mirror the reference's fp32 bezier sampling. [N, S, 2] in pixels."""
    t = np.linspace(0.0, 1.0, NSAMP, dtype=np.float32)[:, None]
    p0, p1, p2, p3 = strokes[:, 0], strokes[:, 1], strokes[:, 2], strokes[:, 3]
    pts = (
        (1 - t[None]) ** 3 * p0[:, None]
        + 3 * (1 - t[None]) ** 2 * t[None] * p1[:, None]
        + 3 * (1 - t[None]) * t[None] ** 2 * p2[:, None]
        + t[None] ** 3 * p3[:, None]
    ).astype(np.float32)
    return pts * np.float32(CS)


def _plan_and_pack(strokes, widths, colors):
    """Host-side pruning, block packing, and input packing.

    Returns (in_maps, plan). Uniform shapes across cores come from per-row
    (8 tiles, one per core) maxima; all shape decisions use row-level data.
    """
    pts = _sample_points(strokes)  # [N,S,2] fp32

    txc = np.arange(NTX, dtype=np.float64) * TW + (TW - 1) / 2.0
    tyc = np.arange(NTY, dtype=np.float64) * TH + (TH - 1) / 2.0
    cx, cy = np.meshgrid(txc, tyc, indexing="xy")
    centers = np.stack([cx.ravel(), cy.ravel()], -1)  # [T,2] f64

    dc = np.sqrt(
        ((centers[:, None, None, :] - pts[None, :, :, :].astype(np.float64)) ** 2).sum(-1)
    )  # [T,N,S]
    dmin_c = dc.min(-1)
    keep = (dc <= dmin_c[:, :, None] + 2 * R_TILE + 0.01) & (
        dc <= R_TILE + widths[None, :, None].astype(np.float64) + W_MARGIN
    )  # [T,N,S]
    k_tn = keep.sum(-1)  # [T,N]
    n_act_t = (k_tn > 0).sum(-1)  # [T]
    k_t = k_tn.max(-1)  # [T]

    cov = np.nonzero(n_act_t > 0)[0]  # covered tiles
    # sort covered tiles by cost: k desc primary, n desc secondary
    order = cov[np.lexsort((-n_act_t[cov], -k_t[cov]))]
    nrows = -(-len(order) // NCORES)
    ntot = nrows * NCORES
    tlist = np.full(ntot, -1, np.int64)  # -1 = dummy tile
    tlist[: len(order)] = order
    rows = tlist.reshape(nrows, NCORES)  # row r, core c -> tile

    # per (row, core): active strokes sorted by candidate count desc
    # slot j of row r: stroke = acts[r][c][j] (or -1)
    acts = []  # [nrows][NCORES] -> int array of stroke ids (len n_act)
    n_row = np.zeros(nrows, np.int64)
    for r in range(nrows):
        row_acts = []
        for c in range(NCORES):
            T = rows[r, c]
            if T < 0:
                row_acts.append(np.empty(0, np.int64))
                continue
            a = np.nonzero(k_tn[T] > 0)[0]
            a = a[np.argsort(-k_tn[T, a], kind="stable")]
            row_acts.append(a)
        acts.append(row_acts)
        n_row[r] = max(1, max(len(a) for a in row_acts))
    # per (row, slot): k = max over cores of that slot's candidate count
    k_slot = []  # [nrows] -> int array [n_row[r]]
    for r in range(nrows):
        ks = np.ones(n_row[r], np.int64)
        for c in range(NCORES):
            T = rows[r, c]
            if T < 0:
                continue
            a = acts[r][c]
            for j in range(len(a)):
                ks[j] = max(ks[j], k_tn[T, a[j]])
        k_slot.append(ks)

    # ---- pack rows into blocks: P = sum n_row <= PMAX, m <= MMAX ----
    blocks = []  # list of list of row indices
    curb, curP = [], 0
    for r in range(nrows):
        if curb and (curP + n_row[r] > PMAX or len(curb) >= MMAX):
            blocks.append(curb)
            curb, curP = [], 0
        curb.append(r)
        curP += int(n_row[r])
    if curb:
        blocks.append(curb)
    NB = len(blocks)

    # block meta: P_b, m_b, row offsets
    blk_P = []
    blk_rows = []
    for b in blocks:
        offs = np.concatenate([[0], np.cumsum([n_row[r] for r in b])])
        blk_P.append(int(offs[-1]))
        blk_rows.append((b, offs))

    # ---- stage-A chunks: per block, runs of stroke-slots with uniform
    # kpad, chunk cols = nstk * kpad <= 512 ----
    # global slot list per block: (row, slot j) with k_slot value
    chunks = []  # (block, s0_in_block, nstk, kpad, col_off)
    col_off = 0
    for bi, (brows, offs) in enumerate(blk_rows):
        slots = []  # flat (k) per stroke-slot in block order
        for r in brows:
            slots.extend(int(k) for k in k_slot[r])
        s = 0
        while s < len(slots):
            kpad = slots[s]
            nstk = 1
            while (
                s + nstk < len(slots)
                and (nstk + 1) * kpad <= 512
            ):
                nstk += 1
            # all strokes in chunk pad to kpad (slots sorted desc per row,
            # but across rows k can rise again; kpad must cover the run)
            kpad = max(slots[s : s + nstk])
            while nstk * kpad > 512:
                nstk -= 1
                kpad = max(slots[s : s + nstk])
            chunks.append((bi, s, nstk, kpad, col_off))
            col_off += nstk * kpad
            s += nstk
    TOTC = col_off

    # ---- mega-groups and chunk-groups ----
    mgs = []  # list of (blk_start, blk_count)
    nmg = -(-NB // MGMAX)
    per = -(-NB // nmg)
    b0 = 0
    while b0 < NB:
        cnt = min(per, NB - b0)
        mgs.append((b0, cnt))
        b0 += cnt
    cgs = []  # list of (mg_idx, blk_start, blk_count, out_idx)
    out_idx = 0
    for gi, (gb0, gcnt) in enumerate(mgs):
        left = gcnt
        pos = gb0
        ncg = -(-gcnt // CGMAX)
        pcg = -(-gcnt // ncg)
        while left > 0:
            take = min(pcg, left)
            cgs.append((gi, pos, take, out_idx))
            out_idx += 1
            pos += take
            left -= take
    NOUT = out_idx
    MAXR = max(3 * len(b) for b, _ in blk_rows)
    OUTW = CGMAX * 128

    # ---- pack per-core tensors ----
    cand = np.zeros((NCORES, 8, max(TOTC, 1)), np.float32)
    cand[:, 4, :] = DUMMY_N  # default: dummy columns
    ucm = np.zeros((NCORES, 128, NB, 256), np.float32)
    w2r = np.zeros((NCORES, 1, NB * 128), np.float32)
    w2c = np.zeros((NCORES, 128, NB), np.float32)
    widths2 = 2.0 * widths.astype(np.float64)
    colors_m1 = colors.astype(np.float64) - 1.0

    # per-core, per-block stroke tables for U'/colors/cand packing
    # row j in block: (row r, slot s) -> core stroke id or -1
    f16 = lambda x: np.float16(x).astype(np.float64)
    for bi, (brows, offs) in enumerate(blk_rows):
        P = blk_P[bi]
        m = len(brows)
        for c in range(NCORES):
            # stroke ids per partition row of this block (this core)
            sid = np.full(P, -1, np.int64)
            tid = np.full(P, -1, np.int64)  # tile-of-row index in block
            for ti, r in enumerate(brows):
                a = acts[r][c]
                o = offs[ti]
                sid[o : o + len(a)] = a
                tid[o : o + int(n_row[r])] = ti
            # U' (suffix + diag) rows/cols [0:P]; w2 in the side tensor
            for s in range(P):
                if sid[s] < 0:
                    continue
                w2r[c, 0, bi * 128 + s] = f16(widths2[sid[s]])
                w2c[c, s, bi] = f16(widths2[sid[s]])
                for j in range(P):
                    if (
                        tid[j] == tid[s]
                        and sid[j] >= 0
                        and (sid[j] > sid[s] or j == s)
                    ):
                        ucm[c, j, bi, s] = -1.0
                # colors at cols [128 + 3*ti : +3]
                ucm[c, s, bi, 128 + 3 * tid[s] : 131 + 3 * tid[s]] = colors_m1[
                    sid[s]
                ]

    # candidates
    for (bi, s0, nstk, kpad, coff) in chunks:
        brows, offs = blk_rows[bi]
        for c in range(NCORES):
            for u in range(nstk):
                s = s0 + u
                # find (row, slot) for block stroke index s
                ti = int(np.searchsorted(offs, s, side="right") - 1)
                r = brows[ti]
                j = s - int(offs[ti])
                T = rows[r, c]
                if T < 0:
                    continue
                a = acts[r][c]
                if j >= len(a):
                    continue
                sidx = int(a[j])
                csamp = np.nonzero(keep[T, sidx])[0]
                q = pts[sidx, csamp].astype(np.float64) - centers[T][None, :]
                qh = f16(q)
                ql = f16(q - qh)
                norm = -((qh[:, 0] + ql[:, 0]) ** 2 + (qh[:, 1] + ql[:, 1]) ** 2)
                nh = f16(norm)
                nl = f16(norm - nh)
                cc = coff + u * kpad
                ncand = len(csamp)
                cand[c, 0, cc : cc + ncand] = 2.0 * qh[:, 0]
                cand[c, 1, cc : cc + ncand] = 2.0 * ql[:, 0]
                cand[c, 2, cc : cc + ncand] = 2.0 * qh[:, 1]
                cand[c, 3, cc : cc + ncand] = 2.0 * ql[:, 1]
                cand[c, 4, cc : cc + ncand] = nh
                cand[c, 5, cc : cc + ncand] = nl
                cand[c, 6, cc : cc + ncand] = -1.0
                # cols [cc+ncand : cc+kpad] stay dummy

    # pixel quad [8, 128]: rows [xl, xl, yl, yl, 1, 1, phi, 0]
    dj = np.tile(np.arange(TW, dtype=np.float64), TH)
    di = np.repeat(np.arange(TH, dtype=np.float64), TW)
    xl = dj - (TW - 1) / 2.0
    yl = di - (TH - 1) / 2.0
    pixq = np.stack(
        [xl, xl, yl, yl, np.ones(128), np.ones(128), xl * xl + yl * yl,
         np.zeros(128)], 0
    )

    ident = np.eye(128, dtype=np.float16)

    in_maps = [
        {
            "cand": cand[c].astype(np.float16),
            "ucm": ucm[c].astype(np.float16),
            "w2r": w2r[c].astype(np.float16),
            "w2c": w2c[c],
            "pixq": pixq.astype(np.float16),
            "ident": ident,
        }
        for c in range(NCORES)
    ]
    plan = {
        "rows": rows,
        "acts": acts,
        "n_row": n_row,
        "blk_rows": blk_rows,
        "blk_P": blk_P,
        "chunks": chunks,
        "mgs": mgs,
        "cgs": cgs,
        "NB": NB,
        "TOTC": TOTC,
        "NOUT": NOUT,
        "MAXR": MAXR,
        "OUTW": OUTW,
        "true_cand": int(k_tn.sum()),
        "ncov": len(order),
    }
    return in_maps, plan


def _build_program(plan, loop_n=None, dynamic_loop=False):
    NB = plan["NB"]
    TOTC = plan["TOTC"]
    NOUT = plan["NOUT"]
    MAXR = plan["MAXR"]
    OUTW = plan["OUTW"]
    blk_P = plan["blk_P"]
    blk_rows = plan["blk_rows"]

    nc = bass.Bass("TRN2", target_bir_lowering=False, debug=False,
                   num_devices=NCORES)
    cand_d = nc.dram_tensor("cand", [8, TOTC], F16, kind="ExternalInput").ap()
    ucm_d = nc.dram_tensor("ucm", [128, NB, 256], F16,
                           kind="ExternalInput").ap()
    w2r_d = nc.dram_tensor("w2r", [1, NB * 128], F16,
                           kind="ExternalInput").ap()
    w2c_d = nc.dram_tensor("w2c", [128, NB], F32, kind="ExternalInput").ap()
    pixq_d = nc.dram_tensor("pixq", [8, 128], F16, kind="ExternalInput").ap()
    ident_d = nc.dram_tensor("ident", [128, 128], F16,
                             kind="ExternalInput").ap()
    out_d = nc.dram_tensor("out", [NOUT, MAXR, OUTW], F32,
                           kind="ExternalOutput").ap()
    niter_d = (
        nc.dram_tensor("niter", [1, 1], mybir.dt.int32, kind="ExternalInput").ap()
        if dynamic_loop
        else None
    )

    with TileContext(nc) as tc:
        with (
            tc.tile_pool(name="const", bufs=1) as constp,
            tc.tile_pool(name="cnd", bufs=2) as cndp,
            tc.tile_pool(name="ucmp", bufs=2) as ucmp,
            tc.tile_pool(name="mbp", bufs=2) as mbp,
            tc.tile_pool(name="sb", bufs=2) as sb,
            tc.tile_pool(name="sbw", bufs=2) as sbw,
            tc.tile_pool(name="outp", bufs=3) as outp,
            tc.tile_pool(name="psdt", bufs=2, space="PSUM") as psdt,
            tc.tile_pool(name="psmt", bufs=2, space="PSUM") as psmt,
            tc.tile_pool(name="pspe", bufs=1, space="PSUM") as pspe,
            tc.tile_pool(name="pspc", bufs=1, space="PSUM") as pspc,
        ):
            pixq_t = constp.tile([8, 128], F16, tag="pixq")
            ident_t = constp.tile([128, 128], F16, tag="ident")
            ones_t = constp.tile([1, 128], F16, tag="ones")
            nc.sync.dma_start(pixq_t[:], pixq_d[:])
            nc.sync.dma_start(ident_t[:], ident_d[:])
            nc.gpsimd.memset(ones_t[:], 1.0)

            import contextlib

            if dynamic_loop:
                nit_t = constp.tile([1, 1], mybir.dt.int32, tag="nit")
                nc.sync.dma_start(nit_t[:], niter_d[:])
                _, (nval,) = nc.values_load_multi_w_load_instructions(
                    nit_t[0:1, 0:1], min_val=1, max_val=8192,
                    skip_runtime_bounds_check=True,
                )
                loop_cm = tc.For_i(0, nval, 1)
            else:
                loop_cm = (
                    tc.For_i(0, loop_n, 1) if loop_n else contextlib.nullcontext()
                )

            with loop_cm:
                # whole-iteration input DMAs
                cand_t = cndp.tile([8, TOTC], F16, tag="cand")
                nc.sync.dma_start(cand_t[:], cand_d[:])
                ucm_t = ucmp.tile([128, NB * 256], F16, tag="ucm")
                nc.sync.dma_start(
                    ucm_t[:], ucm_d[:].rearrange("p b w -> p (b w)")
                )
                w2r_t = ucmp.tile([1, NB * 128], F16, tag="w2r")
                nc.sync.dma_start(w2r_t[:], w2r_d[:])
                w2c_t = ucmp.tile([128, NB], F32, tag="w2c")
                nc.sync.dma_start(w2c_t[:], w2c_d[:])

                for gi, (gb0, gcnt) in enumerate(plan["mgs"]):
                    gw = gcnt * 128
                    mb = mbp.tile([128, MGMAX * 128], F16, tag="mb")
                    mT = psmt.tile([128, MGMAX * 128], F16, tag="mT")
                    # --- stage A: matmul + per-stroke max-reduce ---
                    for (bi, s0, nstk, kpad, coff) in plan["chunks"]:
                        if not (gb0 <= bi < gb0 + gcnt):
                            continue
                        w = nstk * kpad
                        dt = psdt.tile([128, 512], F32, tag="dt")
                        nc.tensor.matmul(
                            dt[:, 0:w], pixq_t[:], cand_t[:, coff : coff + w]
                        )
                        mcol = (bi - gb0) * 128 + s0
                        dt_v = dt[:, 0:w].rearrange("p (n k) -> p n k", n=nstk)
                        nc.vector.tensor_reduce(
                            mb[:, mcol : mcol + nstk],
                            dt_v,
                            axis=mybir.AxisListType.X,
                            op=ALU.max,
                        )
                    # pad columns + clamp on Pool engine
                    for b in range(gcnt):
                        P = blk_P[gb0 + b]
                        if P < 128:
                            nc.gpsimd.memset(
                                mb[:, b * 128 + P : (b + 1) * 128], PAD_MB
                            )
                    nc.gpsimd.tensor_scalar(
                        mb[:, 0:gw], mb[:, 0:gw], -DELTA, None, ALU.min
                    )
                    # transposes: mb [128 pix, 128 strokes] -> mT
                    for b in range(gcnt):
                        sl = slice(b * 128, (b + 1) * 128)
                        nc.tensor.transpose(mT[:, sl], mb[:, sl], ident_t[:])
                    # --- stage B pointwise chain (full mega-group width) ---
                    lnt = sb.tile([128, MGMAX * 128], F32, tag="lnt")
                    s2t = sb.tile([128, MGMAX * 128], F32, tag="s2t")
                    ept = sb.tile([128, MGMAX * 128], F32, tag="ept")
                    spt = sbw.tile([128, MGMAX * 128], F16, tag="spt")
                    argt = sb.tile([128, MGMAX * 128], F32, tag="argt")
                    nc.scalar.activation(
                        lnt[0:PMAX, 0:gw], mT[0:PMAX, 0:gw], AF.Ln, scale=-4.0
                    )
                    nc.scalar.activation(
                        s2t[0:PMAX, 0:gw], lnt[0:PMAX, 0:gw], AF.Exp, scale=0.5
                    )
                    # argt = w2 - 2d  (w2 broadcast per block along columns)
                    w2v = (
                        w2c_t[:, gb0 : gb0 + gcnt]
                        .rearrange("p (b x) -> p b x", x=1)
                        .broadcast_to([128, gcnt, 128])
                    )
                    nc.vector.tensor_tensor(
                        argt[:, 0:gw].rearrange("p (b x) -> p b x", b=gcnt),
                        w2v,
                        s2t[:, 0:gw].rearrange("p (b x) -> p b x", b=gcnt),
                        ALU.subtract,
                    )
                    nc.scalar.activation(
                        ept[0:PMAX, 0:gw], argt[0:PMAX, 0:gw], AF.Exp
                    )
                    nc.scalar.activation(
                        spt[0:PMAX, 0:gw], ept[0:PMAX, 0:gw], AF.Ln, bias=1.0
                    )
                    # --- compositing matmuls per chunk-group ---
                    for (cg_gi, cb0, ccnt, oidx) in plan["cgs"]:
                        if cg_gi != gi:
                            continue
                        pE = pspe.tile([128, CGMAX * 128], F32, tag="pE")
                        pC = pspc.tile([128, CGMAX * 128], F32, tag="pC")
                        t2 = sb.tile([128, CGMAX * 128], F32, tag="t2")
                        wA = sbw.tile([128, CGMAX * 128], F16, tag="wA")
                        outS = outp.tile([MAXR, OUTW], F32, tag="outS")
                        cw = ccnt * 128
                        for b in range(ccnt):
                            bi = cb0 + b
                            P = blk_P[bi]
                            gc = (bi - gb0) * 128  # col in mega-group tiles
                            lc = b * 128  # col in chunk-group tiles
                            nc.tensor.matmul(
                                pE[0:P, lc : lc + 128],
                                ucm_t[0:P, bi * 256 : bi * 256 + P],
                                spt[0:P, gc : gc + 128],
                                start=True, stop=False,
                            )
                            nc.tensor.matmul(
                                pE[0:P, lc : lc + 128],
                                w2r_t[0:1, bi * 128 : bi * 128 + P],
                                ones_t[0:1, :],
                                start=False, stop=True,
                            )
                        mgsl = slice((cb0 - gb0) * 128, (cb0 - gb0) * 128 + cw)
                        nc.vector.tensor_tensor(
                            t2[0:PMAX, 0:cw],
                            pE[0:PMAX, 0:cw],
                            s2t[0:PMAX, mgsl],
                            ALU.subtract,
                        )
                        nc.scalar.activation(
                            wA[0:PMAX, 0:cw], t2[0:PMAX, 0:cw], AF.Exp
                        )
                        for b in range(ccnt):
                            bi = cb0 + b
                            P = blk_P[bi]
                            m = len(blk_rows[bi][0])
                            lc = b * 128
                            nc.tensor.matmul(
                                pC[0 : 3 * m, lc : lc + 128],
                                ucm_t[0:P, bi * 256 + 128 : bi * 256 + 128 + 3 * m],
                                wA[0:P, lc : lc + 128],
                            )
                        nc.scalar.activation(
                            outS[:, 0:cw], pC[0:MAXR, 0:cw], AF.Identity,
                            bias=1.0,
                        )
                        nc.scalar.dma_start(
                            out_d[oidx, :, 0:cw], outS[:, 0:cw]
                        )

    _split_excess_waits(nc)
    return nc


def _scatter(plan, core_outs):
    """Assemble per-core outputs into the [1,3,512,512] canvas."""
    canvas = np.ones((3, CS, CS), np.float32)
    rows = plan["rows"]
    blk_rows = plan["blk_rows"]
    for (gi, cb0, ccnt, oidx) in plan["cgs"]:
        for b in range(ccnt):
            bi = cb0 + b
            brows, offs = blk_rows[bi]
            for ti, r in enumerate(brows):
                for c in range(NCORES):
                    T = int(rows[r, c])
                    if T < 0:
                        continue
                    tyi, txi = divmod(T, NTX)
                    blk = core_outs[c][
                        oidx, 3 * ti : 3 * ti + 3, b * 128 : b * 128 + 128
                    ]
                    canvas[
                        :, tyi * TH : (tyi + 1) * TH, txi * TW : (txi + 1) * TW
                    ] = blk.reshape(3, TH, TW)
    return canvas[None]


def _run(inputs):
    strokes = np.asarray(inputs["strokes"], np.float32)
    widths = np.asarray(inputs["stroke_widths"], np.float32)
    colors = np.asarray(inputs["stroke_colors"], np.float32)
    assert int(inputs["canvas_size"]) == CS

    in_maps, plan = _plan_and_pack(strokes, widths, colors)
    nc = _build_program(plan)
    res = run_bass_kernel_spmd(nc, in_maps, list(range(NCORES)))
    outs = [res.results[c]["out"] for c in range(NCORES)]
    return _scatter(plan, outs), plan, nc, in_maps


def kernel(**inputs):
    out, _, _, _ = _run(inputs)
    return out


def _make_exec(nc, in_maps):
    import jax
    import jax.numpy as jnp
    from jax.sharding import Mesh, PartitionSpec, NamedSharding
    from jax.experimental.shard_map import shard_map
    from concourse import bass2jax

    bass2jax.install_neuronx_cc_hook()
    partition_name = (
        nc.partition_id_tensor.name if nc.partition_id_tensor else None
    )
    in_names, out_names, out_avals = [], [], []
    for alloc in nc.m.functions[0].allocations:
        if not isinstance(alloc, mybir.MemoryLocationSet):
            continue
        name = alloc.memorylocations[0].name
        if alloc.kind == "ExternalInput":
            if name != partition_name:
                in_names.append(name)
        elif alloc.kind == "ExternalOutput":
            out_names.append(name)
            out_avals.append(
                jax.core.ShapedArray(
                    tuple(alloc.tensor_shape), mybir.dt.np(alloc.dtype)
                )
            )
    n_params = len(in_names)
    all_names = in_names + out_names
    if partition_name is not None:
        all_names = all_names + [partition_name]

    def _body(*args):
        operands = list(args)
        if partition_name is not None:
            operands.append(bass2jax.partition_id_tensor())
        outs = bass2jax._bass_exec_p.bind(
            *operands,
            out_avals=tuple(out_avals),
            in_names=tuple(all_names),
            out_names=tuple(out_names),
            lowering_input_output_aliases=(),
            sim_require_finite=True,
            sim_require_nnan=True,
            nc=nc,
        )
        return tuple(outs)

    devices = jax.devices()[:NCORES]
    mesh = Mesh(np.asarray(devices), ("core",))
    n_outs = len(out_names)
    sharded = jax.jit(
        shard_map(
            _body,
            mesh=mesh,
            in_specs=(PartitionSpec("core"),) * (n_params + n_outs),
            out_specs=(PartitionSpec("core"),) * n_outs,
            check_rep=False,
        ),
        donate_argnums=tuple(range(n_params, n_params + n_outs)),
        keep_unused=True,
    )
    concat_in = [
        jnp.asarray(
            np.concatenate([np.asarray(in_maps[c][n]) for c in range(NCORES)], 0)
        )
        for n in in_names
    ]
    zero_shardings = tuple(
        NamedSharding(mesh, PartitionSpec("core")) for _ in out_avals
    )
    zeros_fn = jax.jit(
        lambda: tuple(
            jnp.zeros((a.shape[0] * NCORES,) + a.shape[1:], a.dtype)
            for a in out_avals
        ),
        out_shardings=zero_shardings,
    )

    def run_once():
        return sharded(*concat_in, *zeros_fn())

    return run_once


def timed_run(inputs, reps=10, loop_r=65):
    """Per-iteration device time via runtime trip-count For_i."""
    import jax

    strokes = np.asarray(inputs["strokes"], np.float32)
    widths = np.asarray(inputs["stroke_widths"], np.float32)
    colors = np.asarray(inputs["stroke_colors"], np.float32)
    in_maps, plan = _plan_and_pack(strokes, widths, colors)

    nc = _build_program(plan, dynamic_loop=True)

    def _with_niter(n):
        return [{**m, "niter": np.array([[n]], np.int32)} for m in in_maps]

    run1 = _make_exec(nc, _with_niter(1))
    runR = _make_exec(nc, _with_niter(loop_r))

    outs = None
    for _ in range(3):
        outs = run1()
    jax.block_until_ready(outs)
    jax.block_until_ready(runR())

    t1s, tRs = [], []
    for _ in range(reps):
        t0 = time.perf_counter()
        jax.block_until_ready(run1())
        t1s.append(time.perf_counter() - t0)
        t0 = time.perf_counter()
        jax.block_until_ready(runR())
        tRs.append(time.perf_counter() - t0)
    t1 = float(np.median(t1s))
    tR = float(np.median(tRs))
    dt_ns = (tR - t1) / (loop_r - 1) * 1e9
    print(f"  dispatch t1={t1*1e3:.2f}ms tR={tR*1e3:.2f}ms")

    out_global = np.asarray(outs[0])  # [8*NOUT, MAXR, OUTW]
    NOUT = plan["NOUT"]
    core_outs = [out_global[NOUT * c : NOUT * (c + 1)] for c in range(NCORES)]
    canvas = _scatter(plan, core_outs)
    return canvas, dt_ns, plan


if __name__ == "__main__":
    import reference as ref

    inputs = ref.setup_inputs()
    np_inputs = {
        k: np.asarray(v) if not np.isscalar(v) else v for k, v in inputs.items()
    }
    strokes = np.asarray(np_inputs["strokes"], np.float32)
    widths = np.asarray(np_inputs["stroke_widths"], np.float32)
    colors = np.asarray(np_inputs["stroke_colors"], np.float32)
    t0 = time.time()
    in_maps, plan = _plan_and_pack(strokes, widths, colors)
    print("plan wall:", time.time() - t0)
    print(
        f"NB={plan['NB']} TOTC={plan['TOTC']} true_cand/core~{plan['true_cand']/8:.0f}"
        f" ncov={plan['ncov']} NOUT={plan['NOUT']} MAXR={plan['MAXR']}"
        f" nchunks={len(plan['chunks'])}"
    )
    if os.environ.get("DR_PLANONLY", "0") == "1":
        sys.exit(0)
    t0 = time.time()
    out, plan, nc, in_maps = _run(np_inputs)
    print("kernel wall time:", time.time() - t0)
    expected = np.asarray(ref.reference(**inputs))
    err = np.abs(out - expected)
    print(f"max abs err: {err.max():.3e}  mean: {err.mean():.3e}")


# revision 3
# speedup vs baseline: 2.9135x; 2.9135x over previous
"""Differentiable rasterizer on 8 Trainium2 NeuronCores (Bass/Tile), v2.

Math identical to v1: per pixel/stroke, min over bezier samples of squared
distance via a TensorEngine quadratic form; alpha compositing in closed form
    C = 1 + sum_s alpha_s * T_s * (c_s - 1),  T_s = prod_{j>s} (1 - alpha_j)
in log space with a triangular matmul, using only the ln/exp ACT table:
    2d   = exp(0.5 * ln(-4*m))
    sp   = softplus(2w - 2d) = ln(1 + exp(2w - 2d))
    w_s  = exp((2w - 2d) - sp + psumE) = exp(pE'' - 2d)
where pE'' = (U-I) @ sp + w2 comes from ONE matmul (w2 enters via an
appended ones-row in the rhs and a w2 row in the matrix).

v2 changes for speed:
  - W_MARGIN 46 -> 8 (alpha < sigmoid(-16) ~ 1e-7 beyond; tolerance 2e-2).
    Tiles with no active stroke are skipped on device; host fills 1.0.
  - All matmuls in fp16 (1 PE cycle/row vs fp32's 4). Candidate geometry is
    kept exact with hi/lo split rows (K=8 quadratic form).
  - Variable-depth partition packing: a block stacks as many tiles as fit in
    127 partition rows (strokes) x 128 tile-local pixel columns.
  - Per-chunk candidate padding (chunks are stroke runs with uniform k),
    no segment/max-combine passes.
  - Stage-A per-stroke min clamped to -DELTA on the (idle) Pool engine so
    Ln never sees a non-negative input (pixel exactly on a sample).
"""
import os
import sys
import time

import numpy as np

sys.path.insert(0, "/opt/trn_rl_repo")

import concourse.bass as bass
import concourse.mybir as mybir
from concourse.tile import TileContext
from concourse.bass_utils import run_bass_kernel_spmd

AF = mybir.ActivationFunctionType
ALU = mybir.AluOpType
F32 = mybir.dt.float32
F16 = mybir.dt.float16

CS = 512
NSAMP = 50
NSTR = 64
TH, TW = 8, 16
NTY, NTX = CS // TH, CS // TW
NCORES = 8
R_TILE = float(np.hypot((TW - 1) / 2.0, (TH - 1) / 2.0))
W_MARGIN = 8.0
DELTA = 2.0e-3  # clamp: m <= -DELTA so ln(-4m) is finite
DUMMY_N = -60000.0  # dummy candidate norm row (fits fp16)
PAD_MB = -1000.0  # memset value for mb pad columns

MAX_WAITS = 1
MGMAX = 8  # max blocks per mega-group (mT psum = MGMAX*128 fp16 cols)
CGMAX = 4  # max blocks per stage-B chunk group
PMAX = 128  # strokes per block
MMAX = 42  # tiles per block (3*m <= 126 partition rows in pC)


def _split_excess_waits(nc):
    """walrus rejects >1 sync-wait per instruction; move extras onto NoOps."""
    n_split = 0
    for fn in nc.m.functions:
        for bb in fn.blocks:
            insts = list(bb.instructions)
            out = []
            changed = False
            for inst in insts:
                si = inst.sync_info
                waits = list(si.on_wait) if si is not None and si.on_wait else []
                if len(waits) > MAX_WAITS:
                    changed = True
                    extra = waits[: len(waits) - MAX_WAITS]
                    keep = waits[len(extra):]
                    for i in range(0, len(extra), MAX_WAITS):
                        nop = mybir.InstNoOp(
                            name=f"{inst.name}-ws{n_split}-{i}", ins=[], outs=[]
                        )
                        nop.engine = inst.engine
                        nop.sync_info = mybir.SyncInfo(
                            on_wait=extra[i : i + MAX_WAITS], on_update=[]
                        )
                        out.append(nop)
                    si.on_wait = keep
                    n_split += 1
                out.append(inst)
            if changed:
                bb.instructions[:] = out
    return n_split


def _sample_points(strokes):
    """Mirror the reference's fp32 bezier sampling. [N, S, 2] in pixels."""
    t = np.linspace(0.0, 1.0, NSAMP, dtype=np.float32)[:, None]
    p0, p1, p2, p3 = strokes[:, 0], strokes[:, 1], strokes[:, 2], strokes[:, 3]
    pts = (
        (1 - t[None]) ** 3 * p0[:, None]
        + 3 * (1 - t[None]) ** 2 * t[None] * p1[:, None]
        + 3 * (1 - t[None]) * t[None] ** 2 * p2[:, None]
        + t[None] ** 3 * p3[:, None]
    ).astype(np.float32)
    return pts * np.float32(CS)


def _plan_and_pack(strokes, widths, colors):
    """Host-side pruning, block packing, and input packing.

    Returns (in_maps, plan). Uniform shapes across cores come from per-row
    (8 tiles, one per core) maxima; all shape decisions use row-level data.
    """
    pts = _sample_points(strokes)  # [N,S,2] fp32

    txc = np.arange(NTX, dtype=np.float64) * TW + (TW - 1) / 2.0
    tyc = np.arange(NTY, dtype=np.float64) * TH + (TH - 1) / 2.0
    cx, cy = np.meshgrid(txc, tyc, indexing="xy")
    centers = np.stack([cx.ravel(), cy.ravel()], -1)  # [T,2] f64

    dc = np.sqrt(
        ((centers[:, None, None, :] - pts[None, :, :, :].astype(np.float64)) ** 2).sum(-1)
    )  # [T,N,S]
    dmin_c = dc.min(-1)
    keep = (dc <= dmin_c[:, :, None] + 2 * R_TILE + 0.01) & (
        dc <= R_TILE + widths[None, :, None].astype(np.float64) + W_MARGIN
    )  # [T,N,S]
    k_tn = keep.sum(-1)  # [T,N]
    n_act_t = (k_tn > 0).sum(-1)  # [T]
    k_t = k_tn.max(-1)  # [T]

    cov = np.nonzero(n_act_t > 0)[0]  # covered tiles
    # sort covered tiles by cost: k desc primary, n desc secondary
    order = cov[np.lexsort((-n_act_t[cov], -k_t[cov]))]
    nrows = -(-len(order) // NCORES)
    ntot = nrows * NCORES
    tlist = np.full(ntot, -1, np.int64)  # -1 = dummy tile
    tlist[: len(order)] = order
    rows = tlist.reshape(nrows, NCORES)  # row r, core c -> tile

    # per (row, core): active strokes sorted by candidate count desc
    # slot j of row r: stroke = acts[r][c][j] (or -1)
    acts = []  # [nrows][NCORES] -> int array of stroke ids (len n_act)
    n_row = np.zeros(nrows, np.int64)
    for r in range(nrows):
        row_acts = []
        for c in range(NCORES):
            T = rows[r, c]
            if T < 0:
                row_acts.append(np.empty(0, np.int64))
                continue
            a = np.nonzero(k_tn[T] > 0)[0]
            a = a[np.argsort(-k_tn[T, a], kind="stable")]
            row_acts.append(a)
        acts.append(row_acts)
        n_row[r] = max(1, max(len(a) for a in row_acts))
    # per (row, slot): k = max over cores of that slot's candidate count
    k_slot = []  # [nrows] -> int array [n_row[r]]
    for r in range(nrows):
        ks = np.ones(n_row[r], np.int64)
        for c in range(NCORES):
            T = rows[r, c]
            if T < 0:
                continue
            a = acts[r][c]
            for j in range(len(a)):
                ks[j] = max(ks[j], k_tn[T, a[j]])
        k_slot.append(ks)

    # ---- pack rows into blocks: P = sum n_row <= PMAX, m <= MMAX ----
    blocks = []  # list of list of row indices
    curb, curP = [], 0
    for r in range(nrows):
        if curb and (curP + n_row[r] > PMAX or len(curb) >= MMAX):
            blocks.append(curb)
            curb, curP = [], 0
        curb.append(r)
        curP += int(n_row[r])
    if curb:
        blocks.append(curb)
    NB = len(blocks)

    # block meta: P_b, m_b, row offsets, and the slot permutation.
    # Slot order within a block is free (U'/colors/w2/cand are all packed
    # host-side) -- sort by k desc so chunk kpad padding is tight.
    blk_P = []
    blk_rows = []
    blk_slots = []  # [bi] -> list of (ti, r, j) in partition-row order
    for b in blocks:
        offs = np.concatenate([[0], np.cumsum([n_row[r] for r in b])])
        blk_P.append(int(offs[-1]))
        blk_rows.append((b, offs))
        slots = []
        for ti, r in enumerate(b):
            for j in range(int(n_row[r])):
                slots.append((int(k_slot[r][j]), ti, r, j))
        slots.sort(key=lambda t: -t[0])
        blk_slots.append([(ti, r, j) for (_, ti, r, j) in slots])

    # ---- stage-A chunks: runs of stroke-slots, cols = nstk * kpad <= 512 ----
    chunks = []  # (block, s0_in_block, nstk, kpad, col_off)
    col_off = 0
    for bi, (brows, offs) in enumerate(blk_rows):
        ks = [int(k_slot[r][j]) for (_, r, j) in blk_slots[bi]]
        s = 0
        while s < len(ks):
            kpad = ks[s]  # non-increasing -> kpad = run max
            nstk = 1
            while s + nstk < len(ks) and (nstk + 1) * kpad <= 512:
                nstk += 1
            chunks.append((bi, s, nstk, kpad, col_off))
            col_off += nstk * kpad
            s += nstk
    TOTC = col_off

    # ---- mega-groups and chunk-groups ----
    mgs = []  # list of (blk_start, blk_count)
    nmg = -(-NB // MGMAX)
    per = -(-NB // nmg)
    b0 = 0
    while b0 < NB:
        cnt = min(per, NB - b0)
        mgs.append((b0, cnt))
        b0 += cnt
    cgs = []  # list of (mg_idx, blk_start, blk_count, out_idx)
    out_idx = 0
    for gi, (gb0, gcnt) in enumerate(mgs):
        left = gcnt
        pos = gb0
        ncg = -(-gcnt // CGMAX)
        pcg = -(-gcnt // ncg)
        while left > 0:
            take = min(pcg, left)
            cgs.append((gi, pos, take, out_idx))
            out_idx += 1
            pos += take
            left -= take
    NOUT = out_idx
    MAXR = max(3 * len(b) for b, _ in blk_rows)
    OUTW = CGMAX * 128

    # ---- pack per-core tensors ----
    cand = np.zeros((NCORES, 8, max(TOTC, 1)), np.float32)
    cand[:, 4, :] = DUMMY_N  # default: dummy columns
    ucm = np.zeros((NCORES, 128, NB, 256), np.float32)
    w2r = np.zeros((NCORES, 1, NB * 128), np.float32)
    w2c = np.zeros((NCORES, 128, NB), np.float32)
    widths2 = 2.0 * widths.astype(np.float64)
    colors_m1 = colors.astype(np.float64) - 1.0

    # per-core, per-block stroke tables for U'/colors/cand packing
    # row j in block: (row r, slot s) -> core stroke id or -1
    f16 = lambda x: np.float16(x).astype(np.float64)
    for bi, (brows, offs) in enumerate(blk_rows):
        P = blk_P[bi]
        m = len(brows)
        for c in range(NCORES):
            # stroke ids per partition row (permuted slot order)
            sid = np.full(P, -1, np.int64)
            tid = np.full(P, -1, np.int64)  # tile-of-row index in block
            for p, (ti, r, j) in enumerate(blk_slots[bi]):
                a = acts[r][c]
                if j < len(a):
                    sid[p] = a[j]
                tid[p] = ti
            # U' (suffix + diag) rows/cols [0:P]; w2 in the side tensors
            for s in range(P):
                if sid[s] < 0:
                    continue
                w2r[c, 0, bi * 128 + s] = f16(widths2[sid[s]])
                w2c[c, s, bi] = f16(widths2[sid[s]])
                for j in range(P):
                    if (
                        tid[j] == tid[s]
                        and sid[j] >= 0
                        and (sid[j] > sid[s] or j == s)
                    ):
                        ucm[c, j, bi, s] = -1.0
                # colors at cols [128 + 3*ti : +3]
                ucm[c, s, bi, 128 + 3 * tid[s] : 131 + 3 * tid[s]] = colors_m1[
                    sid[s]
                ]

    # candidates
    for (bi, s0, nstk, kpad, coff) in chunks:
        for c in range(NCORES):
            for u in range(nstk):
                s = s0 + u
                ti, r, j = blk_slots[bi][s]
                T = rows[r, c]
                if T < 0:
                    continue
                a = acts[r][c]
                if j >= len(a):
                    continue
                sidx = int(a[j])
                csamp = np.nonzero(keep[T, sidx])[0]
                q = pts[sidx, csamp].astype(np.float64) - centers[T][None, :]
                qh = f16(q)
                ql = f16(q - qh)
                norm = -((qh[:, 0] + ql[:, 0]) ** 2 + (qh[:, 1] + ql[:, 1]) ** 2)
                nh = f16(norm)
                nl = f16(norm - nh)
                cc = coff + u * kpad
                ncand = len(csamp)
                cand[c, 0, cc : cc + ncand] = 2.0 * qh[:, 0]
                cand[c, 1, cc : cc + ncand] = 2.0 * ql[:, 0]
                cand[c, 2, cc : cc + ncand] = 2.0 * qh[:, 1]
                cand[c, 3, cc : cc + ncand] = 2.0 * ql[:, 1]
                cand[c, 4, cc : cc + ncand] = nh
                cand[c, 5, cc : cc + ncand] = nl
                cand[c, 6, cc : cc + ncand] = -1.0
                # cols [cc+ncand : cc+kpad] stay dummy

    # pixel quad [8, 128]: rows [xl, xl, yl, yl, 1, 1, phi, 0]
    dj = np.tile(np.arange(TW, dtype=np.float64), TH)
    di = np.repeat(np.arange(TH, dtype=np.float64), TW)
    xl = dj - (TW - 1) / 2.0
    yl = di - (TH - 1) / 2.0
    pixq = np.stack(
        [xl, xl, yl, yl, np.ones(128), np.ones(128), xl * xl + yl * yl,
         np.zeros(128)], 0
    )

    ident = np.eye(128, dtype=np.float16)

    in_maps = [
        {
            "cand": cand[c].astype(np.float16),
            "ucm": ucm[c].astype(np.float16),
            "w2r": w2r[c].astype(np.float16),
            "w2c": w2c[c],
            "pixq": pixq.astype(np.float16),
            "ident": ident,
        }
        for c in range(NCORES)
    ]
    plan = {
        "rows": rows,
        "acts": acts,
        "n_row": n_row,
        "blk_rows": blk_rows,
        "blk_P": blk_P,
        "blk_slots": blk_slots,
        "chunks": chunks,
        "mgs": mgs,
        "cgs": cgs,
        "NB": NB,
        "TOTC": TOTC,
        "NOUT": NOUT,
        "MAXR": MAXR,
        "OUTW": OUTW,
        "true_cand": int(k_tn.sum()),
        "ncov": len(order),
    }
    return in_maps, plan


def _build_program(plan, loop_n=None, dynamic_loop=False):
    NB = plan["NB"]
    TOTC = plan["TOTC"]
    NOUT = plan["NOUT"]
    MAXR = plan["MAXR"]
    OUTW = plan["OUTW"]
    blk_P = plan["blk_P"]
    blk_rows = plan["blk_rows"]

    nc = bass.Bass("TRN2", target_bir_lowering=False, debug=False,
                   num_devices=NCORES)
    cand_d = nc.dram_tensor("cand", [8, TOTC], F16, kind="ExternalInput").ap()
    ucm_d = nc.dram_tensor("ucm", [128, NB, 256], F16,
                           kind="ExternalInput").ap()
    w2r_d = nc.dram_tensor("w2r", [1, NB * 128], F16,
                           kind="ExternalInput").ap()
    w2c_d = nc.dram_tensor("w2c", [128, NB], F32, kind="ExternalInput").ap()
    pixq_d = nc.dram_tensor("pixq", [8, 128], F16, kind="ExternalInput").ap()
    ident_d = nc.dram_tensor("ident", [128, 128], F16,
                             kind="ExternalInput").ap()
    out_d = nc.dram_tensor("out", [NOUT, MAXR, OUTW], F32,
                           kind="ExternalOutput").ap()
    niter_d = (
        nc.dram_tensor("niter", [1, 1], mybir.dt.int32, kind="ExternalInput").ap()
        if dynamic_loop
        else None
    )

    with TileContext(nc) as tc:
        with (
            tc.tile_pool(name="const", bufs=1) as constp,
            tc.tile_pool(name="cnd", bufs=2) as cndp,
            tc.tile_pool(name="ucmp", bufs=2) as ucmp,
            tc.tile_pool(name="mbp", bufs=6) as mbp,
            tc.tile_pool(name="sb", bufs=2) as sb,
            tc.tile_pool(name="sbw", bufs=2) as sbw,
            tc.tile_pool(name="outp", bufs=3) as outp,
            tc.tile_pool(name="psdt", bufs=2, space="PSUM") as psdt,
            tc.tile_pool(name="psmt", bufs=2, space="PSUM") as psmt,
            tc.tile_pool(name="pspe", bufs=1, space="PSUM") as pspe,
            tc.tile_pool(name="pspc", bufs=1, space="PSUM") as pspc,
        ):
            pixq_t = constp.tile([8, 128], F16, tag="pixq")
            ident_t = constp.tile([128, 128], F16, tag="ident")
            ones_t = constp.tile([1, 128], F16, tag="ones")
            nc.sync.dma_start(pixq_t[:], pixq_d[:])
            nc.sync.dma_start(ident_t[:], ident_d[:])
            nc.gpsimd.memset(ones_t[:], 1.0)

            import contextlib

            if dynamic_loop:
                nit_t = constp.tile([1, 1], mybir.dt.int32, tag="nit")
                nc.sync.dma_start(nit_t[:], niter_d[:])
                _, (nval,) = nc.values_load_multi_w_load_instructions(
                    nit_t[0:1, 0:1], min_val=1, max_val=8192,
                    skip_runtime_bounds_check=True,
                )
                loop_cm = tc.For_i(0, nval, 1)
            else:
                loop_cm = (
                    tc.For_i(0, loop_n, 1) if loop_n else contextlib.nullcontext()
                )

            with loop_cm:
                # whole-iteration input DMAs
                cand_t = cndp.tile([8, TOTC], F16, tag="cand")
                nc.sync.dma_start(cand_t[:], cand_d[:])
                ucm_t = ucmp.tile([128, NB * 256], F16, tag="ucm")
                nc.sync.dma_start(
                    ucm_t[:], ucm_d[:].rearrange("p b w -> p (b w)")
                )
                w2r_t = ucmp.tile([1, NB * 128], F16, tag="w2r")
                nc.sync.dma_start(w2r_t[:], w2r_d[:])
                w2c_t = ucmp.tile([128, NB], F32, tag="w2c")
                nc.sync.dma_start(w2c_t[:], w2c_d[:])

                for gi, (gb0, gcnt) in enumerate(plan["mgs"]):
                    gw = gcnt * 128
                    mT = psmt.tile([128, MGMAX * 128], F16, tag="mT")
                    # --- stage A: matmul + per-stroke max-reduce, then a
                    # per-block clamp (Pool) + transpose ---
                    for b in range(gcnt):
                        bi = gb0 + b
                        P = blk_P[bi]
                        mb = mbp.tile([128, 128], F16, tag="mb")
                        for (cbi, s0, nstk, kpad, coff) in plan["chunks"]:
                            if cbi != bi:
                                continue
                            w = nstk * kpad
                            dt = psdt.tile([128, 512], F32, tag="dt")
                            nc.tensor.matmul(
                                dt[:, 0:w], pixq_t[:], cand_t[:, coff : coff + w]
                            )
                            dt_v = dt[:, 0:w].rearrange(
                                "p (n k) -> p n k", n=nstk
                            )
                            nc.vector.tensor_reduce(
                                mb[:, s0 : s0 + nstk],
                                dt_v,
                                axis=mybir.AxisListType.X,
                                op=ALU.max,
                            )
                        # clamp so Ln input stays positive (d ~ 0 pixels)
                        nc.gpsimd.tensor_scalar(
                            mb[:, 0:P], mb[:, 0:P], -DELTA, None, ALU.min
                        )
                        sl = slice(b * 128, (b + 1) * 128)
                        nc.tensor.transpose(mT[:, sl], mb[:], ident_t[:])
                    # --- stage B pointwise chain (full mega-group width) ---
                    lnt = sb.tile([128, MGMAX * 128], F32, tag="lnt")
                    s2t = sb.tile([128, MGMAX * 128], F32, tag="s2t")
                    ept = sb.tile([128, MGMAX * 128], F32, tag="ept")
                    spt = sbw.tile([128, MGMAX * 128], F16, tag="spt")
                    argt = sb.tile([128, MGMAX * 128], F32, tag="argt")
                    nc.scalar.activation(
                        lnt[0:PMAX, 0:gw], mT[0:PMAX, 0:gw], AF.Ln, scale=-4.0
                    )
                    nc.scalar.activation(
                        s2t[0:PMAX, 0:gw], lnt[0:PMAX, 0:gw], AF.Exp, scale=0.5
                    )
                    # argt = w2 - 2d  (w2 broadcast per block along columns)
                    w2v = (
                        w2c_t[:, gb0 : gb0 + gcnt]
                        .rearrange("p (b x) -> p b x", x=1)
                        .broadcast_to([128, gcnt, 128])
                    )
                    nc.vector.tensor_tensor(
                        argt[:, 0:gw].rearrange("p (b x) -> p b x", b=gcnt),
                        w2v,
                        s2t[:, 0:gw].rearrange("p (b x) -> p b x", b=gcnt),
                        ALU.subtract,
                    )
                    nc.scalar.activation(
                        ept[0:PMAX, 0:gw], argt[0:PMAX, 0:gw], AF.Exp
                    )
                    nc.scalar.activation(
                        spt[0:PMAX, 0:gw], ept[0:PMAX, 0:gw], AF.Ln, bias=1.0
                    )
                    # --- compositing matmuls per chunk-group ---
                    for (cg_gi, cb0, ccnt, oidx) in plan["cgs"]:
                        if cg_gi != gi:
                            continue
                        pE = pspe.tile([128, CGMAX * 128], F32, tag="pE")
                        pC = pspc.tile([128, CGMAX * 128], F32, tag="pC")
                        t2 = sb.tile([128, CGMAX * 128], F32, tag="t2")
                        wA = sbw.tile([128, CGMAX * 128], F16, tag="wA")
                        outS = outp.tile([MAXR, OUTW], F32, tag="outS")
                        cw = ccnt * 128
                        for b in range(ccnt):
                            bi = cb0 + b
                            P = blk_P[bi]
                            gc = (bi - gb0) * 128  # col in mega-group tiles
                            lc = b * 128  # col in chunk-group tiles
                            nc.tensor.matmul(
                                pE[0:P, lc : lc + 128],
                                ucm_t[0:P, bi * 256 : bi * 256 + P],
                                spt[0:P, gc : gc + 128],
                                start=True, stop=False,
                            )
                            nc.tensor.matmul(
                                pE[0:P, lc : lc + 128],
                                w2r_t[0:1, bi * 128 : bi * 128 + P],
                                ones_t[0:1, :],
                                start=False, stop=True,
                            )
                        mgsl = slice((cb0 - gb0) * 128, (cb0 - gb0) * 128 + cw)
                        nc.vector.tensor_tensor(
                            t2[0:PMAX, 0:cw],
                            pE[0:PMAX, 0:cw],
                            s2t[0:PMAX, mgsl],
                            ALU.subtract,
                        )
                        nc.scalar.activation(
                            wA[0:PMAX, 0:cw], t2[0:PMAX, 0:cw], AF.Exp
                        )
                        for b in range(ccnt):
                            bi = cb0 + b
                            P = blk_P[bi]
                            m = len(blk_rows[bi][0])
                            lc = b * 128
                            nc.tensor.matmul(
                                pC[0 : 3 * m, lc : lc + 128],
                                ucm_t[0:P, bi * 256 + 128 : bi * 256 + 128 + 3 * m],
                                wA[0:P, lc : lc + 128],
                            )
                        nc.scalar.activation(
                            outS[:, 0:cw], pC[0:MAXR, 0:cw], AF.Identity,
                            bias=1.0,
                        )
                        nc.scalar.dma_start(
                            out_d[oidx, :, 0:cw], outS[:, 0:cw]
                        )

    _split_excess_waits(nc)
    return nc


def _scatter(plan, core_outs):
    """Assemble per-core outputs into the [1,3,512,512] canvas."""
    canvas = np.ones((3, CS, CS), np.float32)
    rows = plan["rows"]
    blk_rows = plan["blk_rows"]
    for (gi, cb0, ccnt, oidx) in plan["cgs"]:
        for b in range(ccnt):
            bi = cb0 + b
            brows, offs = blk_rows[bi]
            for ti, r in enumerate(brows):
                for c in range(NCORES):
                    T = int(rows[r, c])
                    if T < 0:
                        continue
                    tyi, txi = divmod(T, NTX)
                    blk = core_outs[c][
                        oidx, 3 * ti : 3 * ti + 3, b * 128 : b * 128 + 128
                    ]
                    canvas[
                        :, tyi * TH : (tyi + 1) * TH, txi * TW : (txi + 1) * TW
                    ] = blk.reshape(3, TH, TW)
    return canvas[None]


def _run(inputs):
    strokes = np.asarray(inputs["strokes"], np.float32)
    widths = np.asarray(inputs["stroke_widths"], np.float32)
    colors = np.asarray(inputs["stroke_colors"], np.float32)
    assert int(inputs["canvas_size"]) == CS

    in_maps, plan = _plan_and_pack(strokes, widths, colors)
    nc = _build_program(plan)
    res = run_bass_kernel_spmd(nc, in_maps, list(range(NCORES)))
    outs = [res.results[c]["out"] for c in range(NCORES)]
    return _scatter(plan, outs), plan, nc, in_maps


def kernel(**inputs):
    out, _, _, _ = _run(inputs)
    return out


def _make_exec(nc, in_maps):
    import jax
    import jax.numpy as jnp
    from jax.sharding import Mesh, PartitionSpec, NamedSharding
    from jax.experimental.shard_map import shard_map
    from concourse import bass2jax

    bass2jax.install_neuronx_cc_hook()
    partition_name = (
        nc.partition_id_tensor.name if nc.partition_id_tensor else None
    )
    in_names, out_names, out_avals = [], [], []
    for alloc in nc.m.functions[0].allocations:
        if not isinstance(alloc, mybir.MemoryLocationSet):
            continue
        name = alloc.memorylocations[0].name
        if alloc.kind == "ExternalInput":
            if name != partition_name:
                in_names.append(name)
        elif alloc.kind == "ExternalOutput":
            out_names.append(name)
            out_avals.append(
                jax.core.ShapedArray(
                    tuple(alloc.tensor_shape), mybir.dt.np(alloc.dtype)
                )
            )
    n_params = len(in_names)
    all_names = in_names + out_names
    if partition_name is not None:
        all_names = all_names + [partition_name]

    def _body(*args):
        operands = list(args)
        if partition_name is not None:
            operands.append(bass2jax.partition_id_tensor())
        outs = bass2jax._bass_exec_p.bind(
            *operands,
            out_avals=tuple(out_avals),
            in_names=tuple(all_names),
            out_names=tuple(out_names),
            lowering_input_output_aliases=(),
            sim_require_finite=True,
            sim_require_nnan=True,
            nc=nc,
        )
        return tuple(outs)

    devices = jax.devices()[:NCORES]
    mesh = Mesh(np.asarray(devices), ("core",))
    n_outs = len(out_names)
    sharded = jax.jit(
        shard_map(
            _body,
            mesh=mesh,
            in_specs=(PartitionSpec("core"),) * (n_params + n_outs),
            out_specs=(PartitionSpec("core"),) * n_outs,
            check_rep=False,
        ),
        donate_argnums=tuple(range(n_params, n_params + n_outs)),
        keep_unused=True,
    )
    concat_in = [
        jnp.asarray(
            np.concatenate([np.asarray(in_maps[c][n]) for c in range(NCORES)], 0)
        )
        for n in in_names
    ]
    zero_shardings = tuple(
        NamedSharding(mesh, PartitionSpec("core")) for _ in out_avals
    )
    zeros_fn = jax.jit(
        lambda: tuple(
            jnp.zeros((a.shape[0] * NCORES,) + a.shape[1:], a.dtype)
            for a in out_avals
        ),
        out_shardings=zero_shardings,
    )

    def run_once():
        return sharded(*concat_in, *zeros_fn())

    return run_once


def timed_run(inputs, reps=10, loop_r=65):
    """Per-iteration device time via runtime trip-count For_i."""
    import jax

    strokes = np.asarray(inputs["strokes"], np.float32)
    widths = np.asarray(inputs["stroke_widths"], np.float32)
    colors = np.asarray(inputs["stroke_colors"], np.float32)
    in_maps, plan = _plan_and_pack(strokes, widths, colors)

    nc = _build_program(plan, dynamic_loop=True)

    def _with_niter(n):
        return [{**m, "niter": np.array([[n]], np.int32)} for m in in_maps]

    run1 = _make_exec(nc, _with_niter(1))
    runR = _make_exec(nc, _with_niter(loop_r))

    outs = None
    for _ in range(3):
        outs = run1()
    jax.block_until_ready(outs)
    jax.block_until_ready(runR())

    t1s, tRs = [], []
    for _ in range(reps):
        t0 = time.perf_counter()
        jax.block_until_ready(run1())
        t1s.append(time.perf_counter() - t0)
        t0 = time.perf_counter()
        jax.block_until_ready(runR())
        tRs.append(time.perf_counter() - t0)
    t1 = float(np.median(t1s))
    tR = float(np.median(tRs))
    dt_ns = (tR - t1) / (loop_r - 1) * 1e9
    print(f"  dispatch t1={t1*1e3:.2f}ms tR={tR*1e3:.2f}ms")

    out_global = np.asarray(outs[0])  # [8*NOUT, MAXR, OUTW]
    NOUT = plan["NOUT"]
    core_outs = [out_global[NOUT * c : NOUT * (c + 1)] for c in range(NCORES)]
    canvas = _scatter(plan, core_outs)
    return canvas, dt_ns, plan


if __name__ == "__main__":
    import reference as ref

    inputs = ref.setup_inputs()
    np_inputs = {
        k: np.asarray(v) if not np.isscalar(v) else v for k, v in inputs.items()
    }
    strokes = np.asarray(np_inputs["strokes"], np.float32)
    widths = np.asarray(np_inputs["stroke_widths"], np.float32)
    colors = np.asarray(np_inputs["stroke_colors"], np.float32)
    t0 = time.time()
    in_maps, plan = _plan_and_pack(strokes, widths, colors)
    print("plan wall:", time.time() - t0)
    print(
        f"NB={plan['NB']} TOTC={plan['TOTC']} true_cand/core~{plan['true_cand']/8:.0f}"
        f" ncov={plan['ncov']} NOUT={plan['NOUT']} MAXR={plan['MAXR']}"
        f" nchunks={len(plan['chunks'])}"
    )
    if os.environ.get("DR_PLANONLY", "0") == "1":
        sys.exit(0)
    t0 = time.time()
    out, plan, nc, in_maps = _run(np_inputs)
    print("kernel wall time:", time.time() - t0)
    expected = np.asarray(ref.reference(**inputs))
    err = np.abs(out - expected)
    print(f"max abs err: {err.max():.3e}  mean: {err.mean():.3e}")


# revision 4
# speedup vs baseline: 6.9151x; 2.3734x over previous
"""Differentiable rasterizer on 8 Trainium2 NeuronCores (Bass/Tile), v2.

Math identical to v1: per pixel/stroke, min over bezier samples of squared
distance via a TensorEngine quadratic form; alpha compositing in closed form
    C = 1 + sum_s alpha_s * T_s * (c_s - 1),  T_s = prod_{j>s} (1 - alpha_j)
in log space with a triangular matmul, using only the ln/exp ACT table:
    2d   = exp(0.5 * ln(-4*m))
    sp   = softplus(2w - 2d) = ln(1 + exp(2w - 2d))
    w_s  = exp((2w - 2d) - sp + psumE) = exp(pE'' - 2d)
where pE'' = (U-I) @ sp + w2 comes from ONE matmul (w2 enters via an
appended ones-row in the rhs and a w2 row in the matrix).

v2 changes for speed:
  - W_MARGIN 46 -> 8 (alpha < sigmoid(-16) ~ 1e-7 beyond; tolerance 2e-2).
    Tiles with no active stroke are skipped on device; host fills 1.0.
  - All matmuls in fp16 (1 PE cycle/row vs fp32's 4). Candidate geometry is
    kept exact with hi/lo split rows (K=8 quadratic form).
  - Variable-depth partition packing: a block stacks as many tiles as fit in
    127 partition rows (strokes) x 128 tile-local pixel columns.
  - Per-chunk candidate padding (chunks are stroke runs with uniform k),
    no segment/max-combine passes.
  - Stage-A per-stroke min clamped to -DELTA on the (idle) Pool engine so
    Ln never sees a non-negative input (pixel exactly on a sample).
"""
import os
import sys
import time

import numpy as np

sys.path.insert(0, "/opt/trn_rl_repo")

import concourse.bass as bass
import concourse.mybir as mybir
from concourse.tile import TileContext
from concourse.bass_utils import run_bass_kernel_spmd

AF = mybir.ActivationFunctionType
ALU = mybir.AluOpType
F32 = mybir.dt.float32
F16 = mybir.dt.float16

CS = 512
NSAMP = 50
NSTR = 64
TH, TW = 8, 16
NTY, NTX = CS // TH, CS // TW
NCORES = 8
R_TILE = float(np.hypot((TW - 1) / 2.0, (TH - 1) / 2.0))
W_MARGIN = 8.0
DELTA = 3.0e-3  # Ln bias: ln(4*(d^2 + DELTA)) stays finite at d = 0
DUMMY_N = -60000.0  # dummy candidate norm row (fits fp16)
PAD_MB = -1000.0  # replica pad value

MAX_WAITS = 1
MGMAX = 4  # max blocks per mega-group (mT psum = MGMAX*128 fp16 cols)
CGMAX = 4  # max blocks per stage-B chunk group
PMAX = 128  # strokes per block
MMAX = 42  # tiles per block (3*m <= 126 partition rows in pC)


def _split_excess_waits(nc):
    """walrus rejects >1 sync-wait per instruction; move extras onto NoOps."""
    n_split = 0
    for fn in nc.m.functions:
        for bb in fn.blocks:
            insts = list(bb.instructions)
            out = []
            changed = False
            for inst in insts:
                si = inst.sync_info
                waits = list(si.on_wait) if si is not None and si.on_wait else []
                if len(waits) > MAX_WAITS:
                    changed = True
                    extra = waits[: len(waits) - MAX_WAITS]
                    keep = waits[len(extra):]
                    for i in range(0, len(extra), MAX_WAITS):
                        nop = mybir.InstNoOp(
                            name=f"{inst.name}-ws{n_split}-{i}", ins=[], outs=[]
                        )
                        nop.engine = inst.engine
                        nop.sync_info = mybir.SyncInfo(
                            on_wait=extra[i : i + MAX_WAITS], on_update=[]
                        )
                        out.append(nop)
                    si.on_wait = keep
                    n_split += 1
                out.append(inst)
            if changed:
                bb.instructions[:] = out
    return n_split


def _sample_points(strokes):
    """Mirror the reference's fp32 bezier sampling. [N, S, 2] in pixels."""
    t = np.linspace(0.0, 1.0, NSAMP, dtype=np.float32)[:, None]
    p0, p1, p2, p3 = strokes[:, 0], strokes[:, 1], strokes[:, 2], strokes[:, 3]
    pts = (
        (1 - t[None]) ** 3 * p0[:, None]
        + 3 * (1 - t[None]) ** 2 * t[None] * p1[:, None]
        + 3 * (1 - t[None]) * t[None] ** 2 * p2[:, None]
        + t[None] ** 3 * p3[:, None]
    ).astype(np.float32)
    return pts * np.float32(CS)


def _plan_and_pack(strokes, widths, colors):
    """Host-side pruning, block packing, and input packing.

    Returns (in_maps, plan). Uniform shapes across cores come from per-row
    (8 tiles, one per core) maxima; all shape decisions use row-level data.
    """
    pts = _sample_points(strokes)  # [N,S,2] fp32

    txc = np.arange(NTX, dtype=np.float64) * TW + (TW - 1) / 2.0
    tyc = np.arange(NTY, dtype=np.float64) * TH + (TH - 1) / 2.0
    cx, cy = np.meshgrid(txc, tyc, indexing="xy")
    centers = np.stack([cx.ravel(), cy.ravel()], -1)  # [T,2] f64

    dc = np.sqrt(
        ((centers[:, None, None, :] - pts[None, :, :, :].astype(np.float64)) ** 2).sum(-1)
    )  # [T,N,S]
    dmin_c = dc.min(-1)
    keep = (dc <= dmin_c[:, :, None] + 2 * R_TILE + 0.01) & (
        dc <= R_TILE + widths[None, :, None].astype(np.float64) + W_MARGIN
    )  # [T,N,S]
    k_tn = keep.sum(-1)  # [T,N]
    n_act_t = (k_tn > 0).sum(-1)  # [T]
    k_t = k_tn.max(-1)  # [T]

    cov = np.nonzero(n_act_t > 0)[0]  # covered tiles
    # sort covered tiles by cost: k desc primary, n desc secondary
    order = cov[np.lexsort((-n_act_t[cov], -k_t[cov]))]
    nrows = -(-len(order) // NCORES)
    ntot = nrows * NCORES
    tlist = np.full(ntot, -1, np.int64)  # -1 = dummy tile
    tlist[: len(order)] = order
    rows = tlist.reshape(nrows, NCORES)  # row r, core c -> tile

    # per (row, core): active strokes sorted by candidate count desc
    # slot j of row r: stroke = acts[r][c][j] (or -1)
    acts = []  # [nrows][NCORES] -> int array of stroke ids (len n_act)
    n_row = np.zeros(nrows, np.int64)
    for r in range(nrows):
        row_acts = []
        for c in range(NCORES):
            T = rows[r, c]
            if T < 0:
                row_acts.append(np.empty(0, np.int64))
                continue
            a = np.nonzero(k_tn[T] > 0)[0]
            a = a[np.argsort(-k_tn[T, a], kind="stable")]
            row_acts.append(a)
        acts.append(row_acts)
        n_row[r] = max(1, max(len(a) for a in row_acts))
    # per (row, slot): k = max over cores of that slot's candidate count
    k_slot = []  # [nrows] -> int array [n_row[r]]
    for r in range(nrows):
        ks = np.ones(n_row[r], np.int64)
        for c in range(NCORES):
            T = rows[r, c]
            if T < 0:
                continue
            a = acts[r][c]
            for j in range(len(a)):
                ks[j] = max(ks[j], k_tn[T, a[j]])
        k_slot.append(ks)

    # ---- pack rows into blocks: P = sum n_row <= PMAX, m <= MMAX ----
    blocks = []  # list of list of row indices
    curb, curP = [], 0
    for r in range(nrows):
        if curb and (curP + n_row[r] > PMAX or len(curb) >= MMAX):
            blocks.append(curb)
            curb, curP = [], 0
        curb.append(r)
        curP += int(n_row[r])
    if curb:
        blocks.append(curb)
    NB = len(blocks)

    # block meta: P_b, m_b, row offsets, and the slot permutation.
    # Slot order within a block is free (U'/colors/w2/cand are all packed
    # host-side) -- sort by k desc so chunk kpad padding is tight.
    blk_P = []
    blk_rows = []
    blk_slots = []  # [bi] -> list of (ti, r, j) in partition-row order
    for b in blocks:
        offs = np.concatenate([[0], np.cumsum([n_row[r] for r in b])])
        blk_P.append(int(offs[-1]))
        blk_rows.append((b, offs))
        slots = []
        for ti, r in enumerate(b):
            for j in range(int(n_row[r])):
                slots.append((int(k_slot[r][j]), ti, r, j))
        slots.sort(key=lambda t: -t[0])
        blk_slots.append([(ti, r, j) for (_, ti, r, j) in slots])

    # ---- stage-A chunks: runs of stroke-slots, cols = nstk * kpad <= 512 ----
    chunks = []  # (block, s0_in_block, nstk, kpad, col_off)
    col_off = 0
    for bi, (brows, offs) in enumerate(blk_rows):
        ks = [int(k_slot[r][j]) for (_, r, j) in blk_slots[bi]]
        s = 0
        while s < len(ks):
            kpad = ks[s]  # non-increasing -> kpad = run max
            nstk = 1
            while s + nstk < len(ks) and (nstk + 1) * kpad <= 512:
                nstk += 1
            chunks.append((bi, s, nstk, kpad, col_off))
            col_off += nstk * kpad
            s += nstk
    TOTC = col_off

    # ---- mega-groups and chunk-groups ----
    mgs = []  # list of (blk_start, blk_count)
    nmg = -(-NB // MGMAX)
    per = -(-NB // nmg)
    b0 = 0
    while b0 < NB:
        cnt = min(per, NB - b0)
        mgs.append((b0, cnt))
        b0 += cnt
    cgs = []  # list of (mg_idx, blk_start, blk_count, out_idx)
    out_idx = 0
    for gi, (gb0, gcnt) in enumerate(mgs):
        left = gcnt
        pos = gb0
        ncg = -(-gcnt // CGMAX)
        pcg = -(-gcnt // ncg)
        while left > 0:
            take = min(pcg, left)
            cgs.append((gi, pos, take, out_idx))
            out_idx += 1
            pos += take
            left -= take
    NOUT = out_idx
    MAXR = max(3 * len(b) for b, _ in blk_rows)
    OUTW = CGMAX * 128

    # ---- pack per-core tensors ----
    cand = np.zeros((NCORES, 8, max(TOTC, 1)), np.float32)
    cand[:, 4, :] = DUMMY_N  # default: dummy columns
    ucm = np.zeros((NCORES, 128, NB, 256), np.float32)
    w2r = np.zeros((NCORES, 1, NB * 128), np.float32)
    w2c = np.zeros((NCORES, 128, NB), np.float32)
    widths2 = 2.0 * widths.astype(np.float64)
    colors_m1 = colors.astype(np.float64) - 1.0

    # per-core, per-block stroke tables for U'/colors/cand packing
    # row j in block: (row r, slot s) -> core stroke id or -1
    f16 = lambda x: np.float16(x).astype(np.float64)
    for bi, (brows, offs) in enumerate(blk_rows):
        P = blk_P[bi]
        m = len(brows)
        for c in range(NCORES):
            # stroke ids per partition row (permuted slot order)
            sid = np.full(P, -1, np.int64)
            tid = np.full(P, -1, np.int64)  # tile-of-row index in block
            for p, (ti, r, j) in enumerate(blk_slots[bi]):
                a = acts[r][c]
                if j < len(a):
                    sid[p] = a[j]
                tid[p] = ti
            # U' (suffix + diag) rows/cols [0:P]; w2 in the side tensors
            for s in range(P):
                if sid[s] < 0:
                    continue
                w2r[c, 0, bi * 128 + s] = f16(widths2[sid[s]])
                w2c[c, s, bi] = f16(widths2[sid[s]])
                for j in range(P):
                    if (
                        tid[j] == tid[s]
                        and sid[j] >= 0
                        and (sid[j] > sid[s] or j == s)
                    ):
                        ucm[c, j, bi, s] = -1.0
                # colors at cols [128 + 3*ti : +3]
                ucm[c, s, bi, 128 + 3 * tid[s] : 131 + 3 * tid[s]] = colors_m1[
                    sid[s]
                ]

    # candidates
    for (bi, s0, nstk, kpad, coff) in chunks:
        for c in range(NCORES):
            for u in range(nstk):
                s = s0 + u
                ti, r, j = blk_slots[bi][s]
                T = rows[r, c]
                if T < 0:
                    continue
                a = acts[r][c]
                if j >= len(a):
                    continue
                sidx = int(a[j])
                csamp = np.nonzero(keep[T, sidx])[0]
                q = pts[sidx, csamp].astype(np.float64) - centers[T][None, :]
                qh = f16(q)
                ql = f16(q - qh)
                norm = -((qh[:, 0] + ql[:, 0]) ** 2 + (qh[:, 1] + ql[:, 1]) ** 2)
                nh = f16(norm)
                nl = f16(norm - nh)
                cc = coff + u * kpad
                ncand = len(csamp)
                cand[c, 0, cc : cc + ncand] = 2.0 * qh[:, 0]
                cand[c, 1, cc : cc + ncand] = 2.0 * ql[:, 0]
                cand[c, 2, cc : cc + ncand] = 2.0 * qh[:, 1]
                cand[c, 3, cc : cc + ncand] = 2.0 * ql[:, 1]
                cand[c, 4, cc : cc + ncand] = nh
                cand[c, 5, cc : cc + ncand] = nl
                cand[c, 6, cc : cc + ncand] = -1.0
                # cols [cc+ncand : cc+kpad] stay dummy

    # pixel quad [8, 128]: rows [xl, xl, yl, yl, 1, 1, phi, 0]
    dj = np.tile(np.arange(TW, dtype=np.float64), TH)
    di = np.repeat(np.arange(TH, dtype=np.float64), TW)
    xl = dj - (TW - 1) / 2.0
    yl = di - (TH - 1) / 2.0
    pixq = np.stack(
        [xl, xl, yl, yl, np.ones(128), np.ones(128), xl * xl + yl * yl,
         np.zeros(128)], 0
    )

    ident = np.eye(128, dtype=np.float16)

    in_maps = [
        {
            "cand": cand[c].astype(np.float16),
            "ucm": ucm[c].astype(np.float16),
            "w2r": w2r[c].astype(np.float16),
            "w2c": w2c[c],
            "lnb": np.full((128, 1), 4.0 * DELTA, np.float32),
            "pixq": pixq.astype(np.float16),
            "ident": ident,
        }
        for c in range(NCORES)
    ]
    plan = {
        "rows": rows,
        "acts": acts,
        "n_row": n_row,
        "blk_rows": blk_rows,
        "blk_P": blk_P,
        "blk_slots": blk_slots,
        "chunks": chunks,
        "mgs": mgs,
        "cgs": cgs,
        "NB": NB,
        "TOTC": TOTC,
        "NOUT": NOUT,
        "MAXR": MAXR,
        "OUTW": OUTW,
        "true_cand": int(k_tn.sum()),
        "ncov": len(order),
    }
    return in_maps, plan


def _build_program(plan, loop_n=None, dynamic_loop=False):
    NB = plan["NB"]
    TOTC = plan["TOTC"]
    NOUT = plan["NOUT"]
    MAXR = plan["MAXR"]
    OUTW = plan["OUTW"]
    blk_P = plan["blk_P"]
    blk_rows = plan["blk_rows"]

    nc = bass.Bass("TRN2", target_bir_lowering=False, debug=False,
                   num_devices=NCORES)
    cand_d = nc.dram_tensor("cand", [8, TOTC], F16, kind="ExternalInput").ap()
    ucm_d = nc.dram_tensor("ucm", [128, NB, 256], F16,
                           kind="ExternalInput").ap()
    w2r_d = nc.dram_tensor("w2r", [1, NB * 128], F16,
                           kind="ExternalInput").ap()
    w2c_d = nc.dram_tensor("w2c", [128, NB], F32, kind="ExternalInput").ap()
    lnb_d = nc.dram_tensor("lnb", [128, 1], F32, kind="ExternalInput").ap()
    pixq_d = nc.dram_tensor("pixq", [8, 128], F16, kind="ExternalInput").ap()
    ident_d = nc.dram_tensor("ident", [128, 128], F16,
                             kind="ExternalInput").ap()
    out_d = nc.dram_tensor("out", [NOUT, MAXR, OUTW], F32,
                           kind="ExternalOutput").ap()
    niter_d = (
        nc.dram_tensor("niter", [1, 1], mybir.dt.int32, kind="ExternalInput").ap()
        if dynamic_loop
        else None
    )

    with TileContext(nc) as tc:
        with (
            tc.tile_pool(name="const", bufs=1) as constp,
            tc.tile_pool(name="cnd", bufs=2) as cndp,
            tc.tile_pool(name="ucmp", bufs=2) as ucmp,
            tc.tile_pool(name="mbp", bufs=2) as mbp,
            tc.tile_pool(name="sb", bufs=2) as sb,
            tc.tile_pool(name="sbw", bufs=2) as sbw,
            tc.tile_pool(name="outp", bufs=3) as outp,
            tc.tile_pool(name="psdt", bufs=3, space="PSUM") as psdt,
            tc.tile_pool(name="psmt", bufs=2, space="PSUM") as psmt,
            tc.tile_pool(name="pspe", bufs=1, space="PSUM") as pspe,
            tc.tile_pool(name="pspc", bufs=1, space="PSUM") as pspc,
        ):
            pixq_t = constp.tile([8, 128], F16, tag="pixq")
            ident_t = constp.tile([128, 128], F16, tag="ident")
            ones_t = constp.tile([1, 128], F16, tag="ones")
            lnb_t = constp.tile([128, 1], F32, tag="lnb")
            nc.sync.dma_start(pixq_t[:], pixq_d[:])
            nc.sync.dma_start(ident_t[:], ident_d[:])
            nc.sync.dma_start(lnb_t[:], lnb_d[:])
            nc.gpsimd.memset(ones_t[:], 1.0)

            import contextlib

            if dynamic_loop:
                nit_t = constp.tile([1, 1], mybir.dt.int32, tag="nit")
                nc.sync.dma_start(nit_t[:], niter_d[:])
                _, (nval,) = nc.values_load_multi_w_load_instructions(
                    nit_t[0:1, 0:1], min_val=1, max_val=8192,
                    skip_runtime_bounds_check=True,
                )
                loop_cm = tc.For_i(0, nval, 1)
            else:
                loop_cm = (
                    tc.For_i(0, loop_n, 1) if loop_n else contextlib.nullcontext()
                )

            with loop_cm:
                # whole-iteration input DMAs
                cand_t = cndp.tile([8, TOTC], F16, tag="cand")
                nc.sync.dma_start(cand_t[:], cand_d[:])
                ucm_t = ucmp.tile([128, NB * 256], F16, tag="ucm")
                nc.sync.dma_start(
                    ucm_t[:], ucm_d[:].rearrange("p b w -> p (b w)")
                )
                w2r_t = ucmp.tile([1, NB * 128], F16, tag="w2r")
                nc.sync.dma_start(w2r_t[:], w2r_d[:])
                w2c_t = ucmp.tile([128, NB], F32, tag="w2c")
                nc.sync.dma_start(w2c_t[:], w2c_d[:])

                def stage_a(gi, gb0, gcnt):
                    """distance matmuls + per-stroke max + transpose -> mT."""
                    mT = psmt.tile([128, MGMAX * 128], F16, tag="mT")
                    mb = mbp.tile([128, MGMAX * 128], F16, tag="mb")
                    for (cbi, s0, nstk, kpad, coff) in plan["chunks"]:
                        if not (gb0 <= cbi < gb0 + gcnt):
                            continue
                        w = nstk * kpad
                        dt = psdt.tile([128, 512], F32, tag="dt")
                        nc.tensor.matmul(
                            dt[:, 0:w], pixq_t[:], cand_t[:, coff : coff + w]
                        )
                        dt_v = dt[:, 0:w].rearrange("p (n k) -> p n k", n=nstk)
                        mcol = (cbi - gb0) * 128 + s0
                        nc.vector.tensor_reduce(
                            mb[:, mcol : mcol + nstk],
                            dt_v,
                            axis=mybir.AxisListType.X,
                            op=ALU.max,
                        )
                    for b in range(gcnt):
                        sl = slice(b * 128, (b + 1) * 128)
                        nc.tensor.transpose(mT[:, sl], mb[:, sl], ident_t[:])
                    return mT

                def stage_b(gi, gb0, gcnt, mT):
                    """pointwise chain + compositing matmuls -> out."""
                    gw = gcnt * 128
                    lnt = sb.tile([128, MGMAX * 128], F32, tag="lnt")
                    s2t = sb.tile([128, MGMAX * 128], F32, tag="s2t")
                    ept = sb.tile([128, MGMAX * 128], F32, tag="ept")
                    spt = sbw.tile([128, MGMAX * 128], F16, tag="spt")
                    argt = sb.tile([128, MGMAX * 128], F32, tag="argt")
                    nc.scalar.activation(
                        lnt[0:PMAX, 0:gw], mT[0:PMAX, 0:gw], AF.Ln,
                        scale=-4.0, bias=lnb_t[0:PMAX, 0:1],
                    )
                    nc.scalar.activation(
                        s2t[0:PMAX, 0:gw], lnt[0:PMAX, 0:gw], AF.Exp, scale=0.5
                    )
                    # argt = w2 - 2d  (w2 broadcast per block along columns)
                    w2v = (
                        w2c_t[:, gb0 : gb0 + gcnt]
                        .rearrange("p (b x) -> p b x", x=1)
                        .broadcast_to([128, gcnt, 128])
                    )
                    nc.vector.tensor_tensor(
                        argt[:, 0:gw].rearrange("p (b x) -> p b x", b=gcnt),
                        w2v,
                        s2t[:, 0:gw].rearrange("p (b x) -> p b x", b=gcnt),
                        ALU.subtract,
                    )
                    nc.scalar.activation(
                        ept[0:PMAX, 0:gw], argt[0:PMAX, 0:gw], AF.Exp
                    )
                    nc.scalar.activation(
                        spt[0:PMAX, 0:gw], ept[0:PMAX, 0:gw], AF.Ln, bias=1.0
                    )
                    for (cg_gi, cb0, ccnt, oidx) in plan["cgs"]:
                        if cg_gi != gi:
                            continue
                        pE = pspe.tile([128, CGMAX * 128], F32, tag="pE")
                        pC = pspc.tile([128, CGMAX * 128], F32, tag="pC")
                        t2 = sb.tile([128, CGMAX * 128], F32, tag="t2")
                        wA = sbw.tile([128, CGMAX * 128], F16, tag="wA")
                        outS = outp.tile([MAXR, OUTW], F32, tag="outS")
                        cw = ccnt * 128
                        for b in range(ccnt):
                            bi = cb0 + b
                            P = blk_P[bi]
                            gc = (bi - gb0) * 128  # col in mega-group tiles
                            lc = b * 128  # col in chunk-group tiles
                            nc.tensor.matmul(
                                pE[0:P, lc : lc + 128],
                                ucm_t[0:P, bi * 256 : bi * 256 + P],
                                spt[0:P, gc : gc + 128],
                                start=True, stop=False,
                            )
                            nc.tensor.matmul(
                                pE[0:P, lc : lc + 128],
                                w2r_t[0:1, bi * 128 : bi * 128 + P],
                                ones_t[0:1, :],
                                start=False, stop=True,
                            )
                        mgsl = slice((cb0 - gb0) * 128, (cb0 - gb0) * 128 + cw)
                        nc.vector.tensor_tensor(
                            t2[0:PMAX, 0:cw],
                            pE[0:PMAX, 0:cw],
                            s2t[0:PMAX, mgsl],
                            ALU.subtract,
                        )
                        nc.scalar.activation(
                            wA[0:PMAX, 0:cw], t2[0:PMAX, 0:cw], AF.Exp
                        )
                        for b in range(ccnt):
                            bi = cb0 + b
                            P = blk_P[bi]
                            m = len(blk_rows[bi][0])
                            lc = b * 128
                            nc.tensor.matmul(
                                pC[0 : 3 * m, lc : lc + 128],
                                ucm_t[0:P, bi * 256 + 128 : bi * 256 + 128 + 3 * m],
                                wA[0:P, lc : lc + 128],
                            )
                        nc.scalar.activation(
                            outS[:, 0:cw], pC[0:MAXR, 0:cw], AF.Identity,
                            bias=1.0,
                        )
                        nc.scalar.dma_start(
                            out_d[oidx, :, 0:cw], outS[:, 0:cw]
                        )

                # software pipeline: keep PE busy on stage A of mg g+1 while
                # ACT/DVE run stage B of mg g
                pending = None
                for gi, (gb0, gcnt) in enumerate(plan["mgs"]):
                    cur = (gi, gb0, gcnt, stage_a(gi, gb0, gcnt))
                    if pending is not None:
                        stage_b(*pending[0:3], pending[3])
                    pending = cur
                stage_b(*pending[0:3], pending[3])

    _split_excess_waits(nc)
    return nc


def _scatter(plan, core_outs):
    """Assemble per-core outputs into the [1,3,512,512] canvas."""
    canvas = np.ones((3, CS, CS), np.float32)
    rows = plan["rows"]
    blk_rows = plan["blk_rows"]
    for (gi, cb0, ccnt, oidx) in plan["cgs"]:
        for b in range(ccnt):
            bi = cb0 + b
            brows, offs = blk_rows[bi]
            for ti, r in enumerate(brows):
                for c in range(NCORES):
                    T = int(rows[r, c])
                    if T < 0:
                        continue
                    tyi, txi = divmod(T, NTX)
                    blk = core_outs[c][
                        oidx, 3 * ti : 3 * ti + 3, b * 128 : b * 128 + 128
                    ]
                    canvas[
                        :, tyi * TH : (tyi + 1) * TH, txi * TW : (txi + 1) * TW
                    ] = blk.reshape(3, TH, TW)
    return canvas[None]


def _run(inputs):
    strokes = np.asarray(inputs["strokes"], np.float32)
    widths = np.asarray(inputs["stroke_widths"], np.float32)
    colors = np.asarray(inputs["stroke_colors"], np.float32)
    assert int(inputs["canvas_size"]) == CS

    in_maps, plan = _plan_and_pack(strokes, widths, colors)
    nc = _build_program(plan)
    res = run_bass_kernel_spmd(nc, in_maps, list(range(NCORES)))
    outs = [res.results[c]["out"] for c in range(NCORES)]
    return _scatter(plan, outs), plan, nc, in_maps


def kernel(**inputs):
    out, _, _, _ = _run(inputs)
    return out


def _make_exec(nc, in_maps):
    import jax
    import jax.numpy as jnp
    from jax.sharding import Mesh, PartitionSpec, NamedSharding
    from jax.experimental.shard_map import shard_map
    from concourse import bass2jax

    bass2jax.install_neuronx_cc_hook()
    partition_name = (
        nc.partition_id_tensor.name if nc.partition_id_tensor else None
    )
    in_names, out_names, out_avals = [], [], []
    for alloc in nc.m.functions[0].allocations:
        if not isinstance(alloc, mybir.MemoryLocationSet):
            continue
        name = alloc.memorylocations[0].name
        if alloc.kind == "ExternalInput":
            if name != partition_name:
                in_names.append(name)
        elif alloc.kind == "ExternalOutput":
            out_names.append(name)
            out_avals.append(
                jax.core.ShapedArray(
                    tuple(alloc.tensor_shape), mybir.dt.np(alloc.dtype)
                )
            )
    n_params = len(in_names)
    all_names = in_names + out_names
    if partition_name is not None:
        all_names = all_names + [partition_name]

    def _body(*args):
        operands = list(args)
        if partition_name is not None:
            operands.append(bass2jax.partition_id_tensor())
        outs = bass2jax._bass_exec_p.bind(
            *operands,
            out_avals=tuple(out_avals),
            in_names=tuple(all_names),
            out_names=tuple(out_names),
            lowering_input_output_aliases=(),
            sim_require_finite=True,
            sim_require_nnan=True,
            nc=nc,
        )
        return tuple(outs)

    devices = jax.devices()[:NCORES]
    mesh = Mesh(np.asarray(devices), ("core",))
    n_outs = len(out_names)
    sharded = jax.jit(
        shard_map(
            _body,
            mesh=mesh,
            in_specs=(PartitionSpec("core"),) * (n_params + n_outs),
            out_specs=(PartitionSpec("core"),) * n_outs,
            check_rep=False,
        ),
        donate_argnums=tuple(range(n_params, n_params + n_outs)),
        keep_unused=True,
    )
    concat_in = [
        jnp.asarray(
            np.concatenate([np.asarray(in_maps[c][n]) for c in range(NCORES)], 0)
        )
        for n in in_names
    ]
    zero_shardings = tuple(
        NamedSharding(mesh, PartitionSpec("core")) for _ in out_avals
    )
    zeros_fn = jax.jit(
        lambda: tuple(
            jnp.zeros((a.shape[0] * NCORES,) + a.shape[1:], a.dtype)
            for a in out_avals
        ),
        out_shardings=zero_shardings,
    )

    def run_once():
        return sharded(*concat_in, *zeros_fn())

    return run_once


def timed_run(inputs, reps=10, loop_r=65):
    """Per-iteration device time via runtime trip-count For_i."""
    import jax

    strokes = np.asarray(inputs["strokes"], np.float32)
    widths = np.asarray(inputs["stroke_widths"], np.float32)
    colors = np.asarray(inputs["stroke_colors"], np.float32)
    in_maps, plan = _plan_and_pack(strokes, widths, colors)

    nc = _build_program(plan, dynamic_loop=True)

    def _with_niter(n):
        return [{**m, "niter": np.array([[n]], np.int32)} for m in in_maps]

    run1 = _make_exec(nc, _with_niter(1))
    runR = _make_exec(nc, _with_niter(loop_r))

    outs = None
    for _ in range(3):
        outs = run1()
    jax.block_until_ready(outs)
    jax.block_until_ready(runR())

    t1s, tRs = [], []
    for _ in range(reps):
        t0 = time.perf_counter()
        jax.block_until_ready(run1())
        t1s.append(time.perf_counter() - t0)
        t0 = time.perf_counter()
        jax.block_until_ready(runR())
        tRs.append(time.perf_counter() - t0)
    t1 = float(np.median(t1s))
    tR = float(np.median(tRs))
    dt_ns = (tR - t1) / (loop_r - 1) * 1e9
    print(f"  dispatch t1={t1*1e3:.2f}ms tR={tR*1e3:.2f}ms")

    out_global = np.asarray(outs[0])  # [8*NOUT, MAXR, OUTW]
    NOUT = plan["NOUT"]
    core_outs = [out_global[NOUT * c : NOUT * (c + 1)] for c in range(NCORES)]
    canvas = _scatter(plan, core_outs)
    return canvas, dt_ns, plan


if __name__ == "__main__":
    import reference as ref

    inputs = ref.setup_inputs()
    np_inputs = {
        k: np.asarray(v) if not np.isscalar(v) else v for k, v in inputs.items()
    }
    strokes = np.asarray(np_inputs["strokes"], np.float32)
    widths = np.asarray(np_inputs["stroke_widths"], np.float32)
    colors = np.asarray(np_inputs["stroke_colors"], np.float32)
    t0 = time.time()
    in_maps, plan = _plan_and_pack(strokes, widths, colors)
    print("plan wall:", time.time() - t0)
    print(
        f"NB={plan['NB']} TOTC={plan['TOTC']} true_cand/core~{plan['true_cand']/8:.0f}"
        f" ncov={plan['ncov']} NOUT={plan['NOUT']} MAXR={plan['MAXR']}"
        f" nchunks={len(plan['chunks'])}"
    )
    if os.environ.get("DR_PLANONLY", "0") == "1":
        sys.exit(0)
    t0 = time.time()
    out, plan, nc, in_maps = _run(np_inputs)
    print("kernel wall time:", time.time() - t0)
    expected = np.asarray(ref.reference(**inputs))
    err = np.abs(out - expected)
    print(f"max abs err: {err.max():.3e}  mean: {err.mean():.3e}")
